# revision 1
# baseline (speedup 1.0000x reference)
"""3-layer GCN (PyG GCNConv semantics) on 8 Trainium2 NeuronCores.

Contract: kernel(**inputs) takes the FULL inputs (x [50000,128] f32,
edge_index [2,800000] int, W1/b1/W2/b2/W3/b3) and returns the FULL
output [50000, 64] f32.

Sharding: nodes are partitioned across the 8 cores by destination
(6250 rows each).  Per layer, each core casts its local rows to fp16 and
an AllGather replicates the full node-feature table to every core's DRAM.
Each core then computes its destination shard:

    out[d,:] = (sum_{edges (s,d)} norm_e * h[s,:]) @ W + b     (+ReLU)

with norm_e = dinv[s]*dinv[d] precomputed per edge on the host (self-loop
edges included).  The sparse aggregation runs as: dma_gather of the 128
source rows of each edge tile (fp16, 256B rows), a one-hot indicator
S[e,d] = norm_e * (d == dst_local_e) built in a single DVE tensor_scalar
(iota is_equal dstloc, then mult norm), and a PE matmul
aggT[f,d] += M[e,f].T @ S[e,d] accumulated in PSUM over the ~17 edge
tiles of each 128-dst block.  dma_gather indices are int16, so the table
is addressed as two halves (<25600 / >=25600 local-row split), which also
lets each half's AllGather fire as soon as the first/second half of the
blocks finished (the per-edge metadata bakes the node->table-row map).
"""

import numpy as np

FEAT = 128
N_CORES = 8
SPLIT_BLK = 25        # blocks 0..24 -> table region A, 25.. -> region B
CHUNK_BLOCKS = 2


# ---------------------------------------------------------------- host side

def preprocess(edge_index: np.ndarray, n_nodes: int, n_cores: int = N_CORES,
               chunk_blocks: int = CHUNK_BLOCKS):
    """Uniform SPMD tile schedule + per-core gather/metadata arrays."""
    src = np.asarray(edge_index[0], dtype=np.int64)
    dst = np.asarray(edge_index[1], dtype=np.int64)
    loops = np.arange(n_nodes, dtype=np.int64)
    s = np.concatenate([src, loops])
    d = np.concatenate([dst, loops])
    deg = np.bincount(d, minlength=n_nodes).astype(np.float64)  # >= 1
    dinv = 1.0 / np.sqrt(deg)
    norm = (dinv[s] * dinv[d]).astype(np.float32)

    S_pc = n_nodes // n_cores
    assert S_pc * n_cores == n_nodes
    nblocks = (S_pc + 127) // 128
    split_blk = min(SPLIT_BLK, (nblocks + 1) // 2)
    ROWS_A = min(split_blk * 128, S_pc)      # local rows in table region A
    ROWS_B = S_pc - ROWS_A
    assert n_cores * ROWS_A < 32768 and n_cores * ROWS_B < 32768

    # node -> table row (region, row): node = q*S_pc + p
    q = s // S_pc
    p = s - q * S_pc
    in_a = p < ROWS_A
    trow = np.where(in_a, q * ROWS_A + p, q * ROWS_B + (p - ROWS_A))

    core_of = d // S_pc
    dloc = d - core_of * S_pc
    blk = dloc // 128
    dst_local = (dloc - blk * 128).astype(np.int32)
    is_hi = (~in_a).astype(np.int64)

    counts = np.zeros((n_cores, nblocks, 2), dtype=np.int64)
    np.add.at(counts, (core_of, blk, is_hi), 1)
    T_lo = -(-counts[:, :, 0].max(axis=0) // 128)
    T_hi = -(-counts[:, :, 1].max(axis=0) // 128)

    order = np.lexsort((is_hi, blk, core_of))
    trow_o, norm_o, dl_o = trow[order], norm[order], dst_local[order]
    co_o, blk_o, hi_o = core_of[order], blk[order], is_hi[order]

    n_tiles_lo = int(T_lo.sum())
    n_tiles_hi = int(T_hi.sum())
    cum_lo = np.concatenate([[0], np.cumsum(T_lo)])
    cum_hi = np.concatenate([[0], np.cumsum(T_hi)])

    key = (co_o * nblocks + blk_o) * 2 + hi_o
    bounds = np.searchsorted(key, np.arange(n_cores * nblocks * 2 + 1))
    per_core = []
    for r in range(n_cores):
        idx_lo = np.zeros(128 * n_tiles_lo, dtype=np.int16)
        idx_hi = np.zeros(128 * n_tiles_hi, dtype=np.int16)
        dloc_lo = np.full((128, max(n_tiles_lo, 1)), -1.0, dtype=np.float32)
        norm_lo = np.zeros((128, max(n_tiles_lo, 1)), dtype=np.float32)
        dloc_hi = np.full((128, max(n_tiles_hi, 1)), -1.0, dtype=np.float32)
        norm_hi = np.zeros((128, max(n_tiles_hi, 1)), dtype=np.float32)
        for b in range(nblocks):
            for h in range(2):
                k = (r * nblocks + b) * 2 + h
                lo_, hi_ = bounds[k], bounds[k + 1]
                cnt = hi_ - lo_
                t0 = cum_lo[b] if h == 0 else cum_hi[b]
                iarr = idx_lo if h == 0 else idx_hi
                darr = dloc_lo if h == 0 else dloc_hi
                narr = norm_lo if h == 0 else norm_hi
                iarr[128 * t0: 128 * t0 + cnt] = trow_o[lo_:hi_].astype(np.int16)
                e = np.arange(cnt)
                darr[e % 128, t0 + e // 128] = dl_o[lo_:hi_]
                narr[e % 128, t0 + e // 128] = norm_o[lo_:hi_]

        def wrap(a):  # [n] -> [128, n//16]; idx i at [i%16 + 16k, i//16]
            n = len(a)
            if n == 0:
                return np.zeros((128, 1), dtype=np.int16)
            w = a.reshape(n // 16, 16).T
            return np.tile(w, (8, 1)).copy()

        per_core.append(dict(
            idx_lo=wrap(idx_lo), idx_hi=wrap(idx_hi),
            dloc_lo=dloc_lo, norm_lo=norm_lo,
            dloc_hi=dloc_hi, norm_hi=norm_hi,
        ))

    chunks = [list(range(c, min(c + chunk_blocks, nblocks)))
              for c in range(0, nblocks, chunk_blocks)]
    sched = dict(
        n_nodes=n_nodes, n_cores=n_cores, S_pc=S_pc, nblocks=nblocks,
        split_blk=split_blk, ROWS_A=ROWS_A, ROWS_B=ROWS_B,
        T_lo=T_lo.astype(int), T_hi=T_hi.astype(int),
        cum_lo=cum_lo.astype(int), cum_hi=cum_hi.astype(int),
        n_tiles_lo=n_tiles_lo, n_tiles_hi=n_tiles_hi, chunks=chunks,
    )
    return sched, per_core


def make_inputs(sched, per_core, x, Ws, bs):
    n_cores, S_pc = sched["n_cores"], sched["S_pc"]
    iota = np.tile(np.arange(128, dtype=np.float32)[None, :], (128, 1))
    in_maps = []
    for r in range(n_cores):
        m = dict(
            x_shard=np.ascontiguousarray(x[r * S_pc:(r + 1) * S_pc]).astype(np.float32),
            iota=iota,
            idx_lo=per_core[r]["idx_lo"], idx_hi=per_core[r]["idx_hi"],
            dloc_lo=per_core[r]["dloc_lo"], norm_lo=per_core[r]["norm_lo"],
            dloc_hi=per_core[r]["dloc_hi"], norm_hi=per_core[r]["norm_hi"],
        )
        for i, (W, b) in enumerate(zip(Ws, bs)):
            m[f"W{i}"] = np.asarray(W).astype(np.float16)
            m[f"b{i}"] = np.tile(np.asarray(b, dtype=np.float32)[None, :], (128, 1))
        in_maps.append(m)
    return in_maps


# ---------------------------------------------------------------- device side

def build_nc(sched, fos=(128, 128, 64), n_cores=None, model=False,
             compile=True, single_packet=False, max_gather_idx=None,
             scratch=None, reps=1, s_cache_tiles=0, probe=None,
             swdge_queues=4, agg_bufs=4):
    """model=True: single-core cost-model variant (AllGather replaced by a
    local DMA) for TimelineSim.  reps>1 replicates the whole pipeline for
    on-hardware delta timing.  s_cache_tiles: number of leading S tiles
    (per half, by global tile index) kept in SBUF and built only on the
    first layer of the first rep."""
    import concourse.bacc as bacc
    import concourse.tile as tile
    import concourse.mybir as mybir

    f16, f32, i16 = mybir.dt.float16, mybir.dt.float32, mybir.dt.int16
    N, S_pc = sched["n_nodes"], sched["S_pc"]
    nblocks = sched["nblocks"]
    ROWS_A = sched["ROWS_A"]
    split_blk = sched["split_blk"]
    T_lo, T_hi = sched["T_lo"], sched["T_hi"]
    cum_lo, cum_hi = sched["cum_lo"], sched["cum_hi"]
    NT_lo, NT_hi = sched["n_tiles_lo"], sched["n_tiles_hi"]
    chunks = sched["chunks"]
    n_cores = n_cores or sched["n_cores"]
    n_layers = len(fos)
    NA = n_cores * ROWS_A            # rows in table region A
    NB = N - NA
    assert NB > 0, "need a non-empty table region B"

    kw = {}
    if scratch:
        kw["dynamic_dma_scratch_size"] = scratch
    if swdge_queues > 1:
        kw["num_swdge_queues"] = swdge_queues
    nc = bacc.Bacc("TRN2", target_bir_lowering=False, debug=False,
                   num_devices=n_cores, **kw)

    x_shard = nc.dram_tensor("x_shard", [S_pc, FEAT], f32, kind="ExternalInput")
    iota_in = nc.dram_tensor("iota", [128, 128], f32, kind="ExternalInput")
    idx_lo_in = nc.dram_tensor("idx_lo", [128, max(NT_lo * 8, 1)], i16, kind="ExternalInput")
    idx_hi_in = nc.dram_tensor("idx_hi", [128, max(NT_hi * 8, 1)], i16, kind="ExternalInput")
    dloc_lo_in = nc.dram_tensor("dloc_lo", [128, max(NT_lo, 1)], f32, kind="ExternalInput")
    norm_lo_in = nc.dram_tensor("norm_lo", [128, max(NT_lo, 1)], f32, kind="ExternalInput")
    dloc_hi_in = nc.dram_tensor("dloc_hi", [128, max(NT_hi, 1)], f32, kind="ExternalInput")
    norm_hi_in = nc.dram_tensor("norm_hi", [128, max(NT_hi, 1)], f32, kind="ExternalInput")
    W_in = [nc.dram_tensor(f"W{i}", [FEAT, fos[i]], f16, kind="ExternalInput")
            for i in range(n_layers)]
    b_in = [nc.dram_tensor(f"b{i}", [128, fos[i]], f32, kind="ExternalInput")
            for i in range(n_layers)]
    y_out = nc.dram_tensor("y", [S_pc, fos[-1]], f32, kind="ExternalOutput")

    rg = [list(range(n_cores))]

    with tile.TileContext(nc) as tc:
        with (
            tc.tile_pool(name="const", bufs=1) as cpool,
            tc.tile_pool(name="sb", bufs=3) as sb,
            tc.tile_pool(name="mbuf", bufs=3) as mbuf,
            tc.tile_pool(name="spool", bufs=6) as spool,
            tc.tile_pool(name="psum_agg", bufs=agg_bufs, space="PSUM") as psum_agg,
            tc.tile_pool(name="psum_mm", bufs=2, space="PSUM") as psum_mm,
            tc.tile_pool(name="dram", bufs=2, space="DRAM") as dram,
        ):
            # --- constants, loaded once
            iota = cpool.tile([128, 128], f32)
            nc.sync.dma_start(out=iota[:], in_=iota_in[:])
            idx_lo = cpool.tile([128, max(NT_lo * 8, 1)], i16)
            nc.sync.dma_start(out=idx_lo[:], in_=idx_lo_in[:])
            idx_hi = cpool.tile([128, max(NT_hi * 8, 1)], i16)
            nc.sync.dma_start(out=idx_hi[:], in_=idx_hi_in[:])
            dloc_lo = cpool.tile([128, max(NT_lo, 1)], f32)
            nc.sync.dma_start(out=dloc_lo[:], in_=dloc_lo_in[:])
            norm_lo = cpool.tile([128, max(NT_lo, 1)], f32)
            nc.sync.dma_start(out=norm_lo[:], in_=norm_lo_in[:])
            dloc_hi = cpool.tile([128, max(NT_hi, 1)], f32)
            nc.sync.dma_start(out=dloc_hi[:], in_=dloc_hi_in[:])
            norm_hi = cpool.tile([128, max(NT_hi, 1)], f32)
            nc.sync.dma_start(out=norm_hi[:], in_=norm_hi_in[:])
            Wt, bt = [], []
            for i in range(n_layers):
                w = cpool.tile([FEAT, fos[i]], f16, tag=f"W{i}")
                nc.sync.dma_start(out=w[:], in_=W_in[i][:])
                Wt.append(w)
                b = cpool.tile([128, fos[i]], f32, tag=f"b{i}")
                nc.sync.dma_start(out=b[:], in_=b_in[i][:])
                bt.append(b)

            # SBUF cache for S tiles reused across layers/reps
            nsc_lo = min(s_cache_tiles, NT_lo)
            nsc_hi = min(s_cache_tiles, NT_hi)
            s_cache = {}
            if nsc_lo + nsc_hi:
                sc = cpool.tile([128, (nsc_lo + nsc_hi) * 128], f16, tag="scache")
                for t in range(nsc_lo):
                    s_cache[(0, t)] = sc[:, t * 128:(t + 1) * 128]
                for t in range(nsc_hi):
                    s_cache[(1, t)] = sc[:, (nsc_lo + t) * 128:(nsc_lo + t + 1) * 128]
            s_built = set()

            def get_S(half, col, dl, nm):
                """S tile for global tile index `col` of half (0=lo,1=hi)."""
                key = (half, col)
                if key in s_cache:
                    if key not in s_built:
                        s_built.add(key)
                        S = s_cache[key]
                        nc.vector.tensor_scalar(
                            S, iota[:], dl[:, col:col + 1], nm[:, col:col + 1],
                            mybir.AluOpType.is_equal, mybir.AluOpType.mult)
                    return s_cache[key]
                if probe == "noS":
                    if "hoisted" not in s_cache:
                        S = spool.tile([128, 128], f16, tag="S")
                        nc.vector.tensor_scalar(
                            S[:], iota[:], dl[:, col:col + 1], nm[:, col:col + 1],
                            mybir.AluOpType.is_equal, mybir.AluOpType.mult)
                        s_cache["hoisted"] = S[:]
                    return s_cache["hoisted"]
                S = spool.tile([128, 128], f16, tag="S")
                nc.vector.tensor_scalar(
                    S[:], iota[:], dl[:, col:col + 1], nm[:, col:col + 1],
                    mybir.AluOpType.is_equal, mybir.AluOpType.mult)
                return S[:]

            gq = [0]

            def emit_gather(M, src_ap, idx_sb, t0, nt):
                if probe == "nogather":
                    nc.sync.dma_start(out=M[:],
                                      in_=src_ap[0:128 * nt, :].rearrange(
                                          "(p t) f -> p t f", p=128))
                    return
                step = nt if not max_gather_idx else max(1, max_gather_idx // 128)
                for s0 in range(0, nt, step):
                    sn = min(step, nt - s0)
                    o16 = (t0 + s0) * 8
                    gq[0] = (gq[0] + 1) % swdge_queues
                    nc.gpsimd.dma_gather(
                        out_ap=M[:, s0:s0 + sn, :], in_ap=src_ap,
                        idxs_ap=idx_sb[:, o16:o16 + sn * 8],
                        num_idxs=128 * sn, num_idxs_reg=128 * sn,
                        elem_size=FEAT, single_packet=single_packet,
                        queue_num=gq[0])

            for rep in range(reps):
                # --- layer 0 input: cast x_shard to fp16 (split A/B bounce)
                ag_in_a = dram.tile([ROWS_A, FEAT], f16, tag="ag_in_a")
                ag_in_b = dram.tile([S_pc - ROWS_A, FEAT], f16, tag="ag_in_b")
                for b in range(nblocks):
                    r0, r1 = b * 128, min((b + 1) * 128, S_pc)
                    rows = r1 - r0
                    xt = sb.tile([128, FEAT], f32, tag="xcast_in")
                    nc.sync.dma_start(out=xt[:rows, :], in_=x_shard[r0:r1, :])
                    xh = sb.tile([128, FEAT], f16, tag="xcast_out")
                    nc.vector.tensor_copy(xh[:rows, :], xt[:rows, :])
                    if b < split_blk:
                        nc.sync.dma_start(out=ag_in_a[r0:r1, :], in_=xh[:rows, :])
                    else:
                        nc.sync.dma_start(
                            out=ag_in_b[r0 - ROWS_A:r1 - ROWS_A, :],
                            in_=xh[:rows, :])

                for l in range(n_layers):
                    fo = fos[l]
                    # --- allgather: region A (blocks < split_blk), region B
                    tbl_a = dram.tile([NA, FEAT], f16, tag="tbl_a")
                    tbl_b = dram.tile([NB, FEAT], f16, tag="tbl_b")
                    if model:
                        nc.sync.dma_start(out=tbl_a[0:ROWS_A, :], in_=ag_in_a[:])
                        nc.sync.dma_start(out=tbl_b[0:S_pc - ROWS_A, :], in_=ag_in_b[:])
                    else:
                        nc.gpsimd.collective_compute(
                            "AllGather", mybir.AluOpType.bypass,
                            replica_groups=rg,
                            ins=[ag_in_a[:].opt()], outs=[tbl_a[:].opt()])
                        nc.gpsimd.collective_compute(
                            "AllGather", mybir.AluOpType.bypass,
                            replica_groups=rg,
                            ins=[ag_in_b[:].opt()], outs=[tbl_b[:].opt()])
                    if l + 1 < n_layers:
                        ag_in_a = dram.tile([ROWS_A, FEAT], f16, tag="ag_in_a")
                        ag_in_b = dram.tile([S_pc - ROWS_A, FEAT], f16, tag="ag_in_b")

                    for chunk in chunks:
                        ctlo = int(sum(T_lo[b] for b in chunk))
                        cthi = int(sum(T_hi[b] for b in chunk))
                        M_lo = M_hi = None
                        if ctlo:
                            M_lo = mbuf.tile([128, ctlo, 128], f16, tag="Mlo")
                            emit_gather(M_lo, tbl_a[:], idx_lo,
                                        int(cum_lo[chunk[0]]), ctlo)
                        if cthi:
                            M_hi = mbuf.tile([128, cthi, 128], f16, tag="Mhi")
                            emit_gather(M_hi, tbl_b[:], idx_hi,
                                        int(cum_hi[chunk[0]]), cthi)
                        for b in chunk:
                            tiles = (
                                [(0, M_lo, cum_lo[b] - cum_lo[chunk[0]] + k,
                                  cum_lo[b] + k, dloc_lo, norm_lo)
                                 for k in range(T_lo[b])] +
                                [(1, M_hi, cum_hi[b] - cum_hi[chunk[0]] + k,
                                  cum_hi[b] + k, dloc_hi, norm_hi)
                                 for k in range(T_hi[b])])
                            agg = psum_agg.tile([128, 128], f32)
                            for j, (half, Mt, sl, col, dl, nm) in enumerate(tiles):
                                S = get_S(half, int(col), dl, nm)
                                nc.tensor.matmul(agg[:], Mt[:, sl, :], S,
                                                 start=(j == 0),
                                                 stop=(j == len(tiles) - 1))
                            aggT = sb.tile([128, 128], f16, tag="aggT")
                            nc.scalar.copy(aggT[:], agg[:])
                            mm = psum_mm.tile([128, fo], f32, tag="mm")
                            nc.tensor.matmul(mm[:], aggT[:], Wt[l][:],
                                             start=True, stop=True)
                            r0, r1 = b * 128, min((b + 1) * 128, S_pc)
                            rows = r1 - r0
                            if l + 1 < n_layers:
                                hs = sb.tile([128, fo], f16, tag="hsum")
                                nc.vector.tensor_tensor(
                                    hs[:], mm[:], bt[l][:], mybir.AluOpType.add)
                                h = sb.tile([128, fo], f16, tag="hout")
                                nc.scalar.activation(
                                    h[:], hs[:], mybir.ActivationFunctionType.Relu)
                                if b < split_blk:
                                    nc.sync.dma_start(out=ag_in_a[r0:r1, :],
                                                      in_=h[:rows, :])
                                else:
                                    nc.sync.dma_start(
                                        out=ag_in_b[r0 - ROWS_A:r1 - ROWS_A, :],
                                        in_=h[:rows, :])
                            else:
                                o = sb.tile([128, fo], f32, tag="oout")
                                nc.vector.tensor_tensor(
                                    o[:], mm[:], bt[l][:], mybir.AluOpType.add)
                                nc.sync.dma_start(out=y_out[r0:r1, :],
                                                  in_=o[:rows, :])
    if compile:
        nc.compile()
    return nc


# ---------------------------------------------------------------- entry point

_CACHE = {}


def kernel(x, edge_index, W1, b1, W2, b2, W3, b3):
    import sys
    if "/opt/trn_rl_repo" not in sys.path:
        sys.path.insert(0, "/opt/trn_rl_repo")
    from concourse import bass_utils

    x = np.asarray(x)
    edge_index = np.asarray(edge_index)
    Ws = [np.asarray(W1), np.asarray(W2), np.asarray(W3)]
    bs = [np.asarray(b1), np.asarray(b2), np.asarray(b3)]
    n = x.shape[0]

    key = (n, edge_index.shape[1])
    if key in _CACHE and np.array_equal(_CACHE[key][0], edge_index):
        _, sched, per_core, nc = _CACHE[key]
    else:
        sched, per_core = preprocess(edge_index, n, N_CORES)
        nc = build_nc(sched, fos=(W1.shape[1], W2.shape[1], W3.shape[1]),
                      s_cache_tiles=SCACHE_TILES)
        _CACHE[key] = (edge_index.copy(), sched, per_core, nc)

    in_maps = make_inputs(sched, per_core, x, Ws, bs)
    res = bass_utils.run_bass_kernel_spmd(nc, in_maps,
                                          core_ids=list(range(N_CORES)))
    out = np.concatenate([res.results[r]["y"] for r in range(N_CORES)], axis=0)
    return out.astype(np.float32)


SCACHE_TILES = 0  # tuned below; number of per-half S tiles cached in SBUF



# revision 6
# speedup vs baseline: 1.1978x; 1.1978x over previous
"""3-layer GCN (PyG GCNConv semantics) on 8 Trainium2 NeuronCores.

Contract: kernel(**inputs) takes the FULL inputs (x [50000,128] f32,
edge_index [2,800000] int, W1/b1/W2/b2/W3/b3) and returns the FULL
output [50000, 64] f32.

Sharding: nodes are partitioned across the 8 cores by destination
(6250 rows each).  Per layer, each core casts its local rows to fp16 and
an AllGather replicates the full node-feature table to every core's DRAM.
Each core then computes its destination shard:

    out[d,:] = (sum_{edges (s,d)} norm_e * h[s,:]) @ W + b     (+ReLU)

with norm_e = dinv[s]*dinv[d] precomputed per edge on the host (self-loop
edges included).  The sparse aggregation runs as: dma_gather of the 128
source rows of each edge tile (fp16, 256B rows), a one-hot indicator
S[e,d] = norm_e * (d == dst_local_e) built in a single DVE tensor_scalar
(iota is_equal dstloc, then mult norm), and a PE matmul
aggT[f,d] += M[e,f].T @ S[e,d] accumulated in PSUM over the ~17 edge
tiles of each 128-dst block.  dma_gather indices are int16, so the table
is addressed as two halves (<25600 / >=25600 local-row split), which also
lets each half's AllGather fire as soon as the first/second half of the
blocks finished (the per-edge metadata bakes the node->table-row map).
"""

import numpy as np

FEAT = 128
N_CORES = 8
SPLIT_BLK = 25        # blocks 0..24 -> table region A, 25.. -> region B
CHUNK_BLOCKS = 2


# ---------------------------------------------------------------- host side

def preprocess(edge_index: np.ndarray, n_nodes: int, n_cores: int = N_CORES,
               chunk_blocks: int = CHUNK_BLOCKS):
    """Uniform SPMD tile schedule + per-core gather/metadata arrays."""
    src = np.asarray(edge_index[0], dtype=np.int64)
    dst = np.asarray(edge_index[1], dtype=np.int64)
    loops = np.arange(n_nodes, dtype=np.int64)
    s = np.concatenate([src, loops])
    d = np.concatenate([dst, loops])
    deg = np.bincount(d, minlength=n_nodes).astype(np.float64)  # >= 1
    dinv = 1.0 / np.sqrt(deg)
    norm = (dinv[s] * dinv[d]).astype(np.float32)

    S_pc = n_nodes // n_cores
    assert S_pc * n_cores == n_nodes
    nblocks = (S_pc + 127) // 128
    split_blk = min(SPLIT_BLK, (nblocks + 1) // 2)
    ROWS_A = min(split_blk * 128, S_pc)      # local rows in table region A
    ROWS_B = S_pc - ROWS_A
    assert n_cores * ROWS_A < 32768 and n_cores * ROWS_B < 32768

    # node -> table row (region, row): node = q*S_pc + p
    q = s // S_pc
    p = s - q * S_pc
    in_a = p < ROWS_A
    trow = np.where(in_a, q * ROWS_A + p, q * ROWS_B + (p - ROWS_A))

    core_of = d // S_pc
    dloc = d - core_of * S_pc
    blk = dloc // 128
    dst_local = (dloc - blk * 128).astype(np.int32)
    is_hi = (~in_a).astype(np.int64)

    counts = np.zeros((n_cores, nblocks, 2), dtype=np.int64)
    np.add.at(counts, (core_of, blk, is_hi), 1)
    T_lo = -(-counts[:, :, 0].max(axis=0) // 128)
    T_hi = -(-counts[:, :, 1].max(axis=0) // 128)

    order = np.lexsort((is_hi, blk, core_of))
    trow_o, norm_o, dl_o = trow[order], norm[order], dst_local[order]
    co_o, blk_o, hi_o = core_of[order], blk[order], is_hi[order]

    n_tiles_lo = int(T_lo.sum())
    n_tiles_hi = int(T_hi.sum())
    cum_lo = np.concatenate([[0], np.cumsum(T_lo)])
    cum_hi = np.concatenate([[0], np.cumsum(T_hi)])

    key = (co_o * nblocks + blk_o) * 2 + hi_o
    bounds = np.searchsorted(key, np.arange(n_cores * nblocks * 2 + 1))
    per_core = []
    for r in range(n_cores):
        idx_lo = np.zeros(128 * n_tiles_lo, dtype=np.int16)
        idx_hi = np.zeros(128 * n_tiles_hi, dtype=np.int16)
        dloc_lo = np.full((128, max(n_tiles_lo, 1)), -1.0, dtype=np.float32)
        norm_lo = np.zeros((128, max(n_tiles_lo, 1)), dtype=np.float32)
        dloc_hi = np.full((128, max(n_tiles_hi, 1)), -1.0, dtype=np.float32)
        norm_hi = np.zeros((128, max(n_tiles_hi, 1)), dtype=np.float32)
        for b in range(nblocks):
            for h in range(2):
                k = (r * nblocks + b) * 2 + h
                lo_, hi_ = bounds[k], bounds[k + 1]
                cnt = hi_ - lo_
                t0 = cum_lo[b] if h == 0 else cum_hi[b]
                iarr = idx_lo if h == 0 else idx_hi
                darr = dloc_lo if h == 0 else dloc_hi
                narr = norm_lo if h == 0 else norm_hi
                iarr[128 * t0: 128 * t0 + cnt] = trow_o[lo_:hi_].astype(np.int16)
                e = np.arange(cnt)
                darr[e % 128, t0 + e // 128] = dl_o[lo_:hi_]
                narr[e % 128, t0 + e // 128] = norm_o[lo_:hi_]

        def wrap(a):  # [n] -> [128, n//16]; idx i at [i%16 + 16k, i//16]
            n = len(a)
            if n == 0:
                return np.zeros((128, 1), dtype=np.int16)
            w = a.reshape(n // 16, 16).T
            return np.tile(w, (8, 1)).copy()

        per_core.append(dict(
            idx_lo=wrap(idx_lo), idx_hi=wrap(idx_hi),
            dloc_lo=dloc_lo, norm_lo=norm_lo,
            dloc_hi=dloc_hi, norm_hi=norm_hi,
        ))

    chunks = [list(range(c, min(c + chunk_blocks, nblocks)))
              for c in range(0, nblocks, chunk_blocks)]
    sched = dict(
        n_nodes=n_nodes, n_cores=n_cores, S_pc=S_pc, nblocks=nblocks,
        split_blk=split_blk, ROWS_A=ROWS_A, ROWS_B=ROWS_B,
        T_lo=T_lo.astype(int), T_hi=T_hi.astype(int),
        cum_lo=cum_lo.astype(int), cum_hi=cum_hi.astype(int),
        n_tiles_lo=n_tiles_lo, n_tiles_hi=n_tiles_hi, chunks=chunks,
    )
    return sched, per_core


def make_inputs(sched, per_core, x, Ws, bs):
    n_cores, S_pc = sched["n_cores"], sched["S_pc"]
    iota = np.tile(np.arange(128, dtype=np.float32)[None, :], (128, 1))
    in_maps = []
    for r in range(n_cores):
        m = dict(
            x_shard=np.ascontiguousarray(x[r * S_pc:(r + 1) * S_pc]).astype(np.float32),
            iota=iota,
            idx_lo=per_core[r]["idx_lo"], idx_hi=per_core[r]["idx_hi"],
            dloc_lo=per_core[r]["dloc_lo"], norm_lo=per_core[r]["norm_lo"],
            dloc_hi=per_core[r]["dloc_hi"], norm_hi=per_core[r]["norm_hi"],
        )
        for i, (W, b) in enumerate(zip(Ws, bs)):
            m[f"W{i}"] = np.asarray(W).astype(np.float16)
            m[f"b{i}"] = np.tile(np.asarray(b, dtype=np.float32)[None, :], (128, 1))
        in_maps.append(m)
    return in_maps


# ---------------------------------------------------------------- device side

def build_nc(sched, fos=(128, 128, 64), n_cores=None, model=False,
             compile=True, single_packet=False, max_gather_idx=None,
             scratch=None, reps=1, s_cache_tiles=0, probe=None,
             swdge_queues=4, agg_bufs=4, shared_cc=True):
    """model=True: single-core cost-model variant (AllGather replaced by a
    local DMA) for TimelineSim.  reps>1 replicates the whole pipeline for
    on-hardware delta timing.  s_cache_tiles: number of leading S tiles
    (per half, by global tile index) kept in SBUF and built only on the
    first layer of the first rep."""
    import concourse.bacc as bacc
    import concourse.tile as tile
    import concourse.mybir as mybir

    f16, f32, i16 = mybir.dt.float16, mybir.dt.float32, mybir.dt.int16
    N, S_pc = sched["n_nodes"], sched["S_pc"]
    nblocks = sched["nblocks"]
    ROWS_A = sched["ROWS_A"]
    split_blk = sched["split_blk"]
    T_lo, T_hi = sched["T_lo"], sched["T_hi"]
    cum_lo, cum_hi = sched["cum_lo"], sched["cum_hi"]
    NT_lo, NT_hi = sched["n_tiles_lo"], sched["n_tiles_hi"]
    chunks = sched["chunks"]
    n_cores = n_cores or sched["n_cores"]
    n_layers = len(fos)
    NA = n_cores * ROWS_A            # rows in table region A
    NB = N - NA
    assert NB > 0, "need a non-empty table region B"

    kw = {}
    if scratch:
        kw["dynamic_dma_scratch_size"] = scratch
    if swdge_queues > 1:
        kw["num_swdge_queues"] = swdge_queues
    nc = bacc.Bacc("TRN2", target_bir_lowering=False, debug=False,
                   num_devices=n_cores, **kw)

    x_shard = nc.dram_tensor("x_shard", [S_pc, FEAT], f32, kind="ExternalInput")
    iota_in = nc.dram_tensor("iota", [128, 128], f32, kind="ExternalInput")
    idx_lo_in = nc.dram_tensor("idx_lo", [128, max(NT_lo * 8, 1)], i16, kind="ExternalInput")
    idx_hi_in = nc.dram_tensor("idx_hi", [128, max(NT_hi * 8, 1)], i16, kind="ExternalInput")
    dloc_lo_in = nc.dram_tensor("dloc_lo", [128, max(NT_lo, 1)], f32, kind="ExternalInput")
    norm_lo_in = nc.dram_tensor("norm_lo", [128, max(NT_lo, 1)], f32, kind="ExternalInput")
    dloc_hi_in = nc.dram_tensor("dloc_hi", [128, max(NT_hi, 1)], f32, kind="ExternalInput")
    norm_hi_in = nc.dram_tensor("norm_hi", [128, max(NT_hi, 1)], f32, kind="ExternalInput")
    W_in = [nc.dram_tensor(f"W{i}", [FEAT, fos[i]], f16, kind="ExternalInput")
            for i in range(n_layers)]
    b_in = [nc.dram_tensor(f"b{i}", [128, fos[i]], f32, kind="ExternalInput")
            for i in range(n_layers)]
    y_out = nc.dram_tensor("y", [S_pc, fos[-1]], f32, kind="ExternalOutput")

    rg = [list(range(n_cores))]

    with tile.TileContext(nc) as tc:
        with (
            tc.tile_pool(name="const", bufs=1) as cpool,
            tc.tile_pool(name="sb", bufs=3) as sb,
            tc.tile_pool(name="mbuf", bufs=3) as mbuf,
            tc.tile_pool(name="spool", bufs=6) as spool,
            tc.tile_pool(name="psum_agg", bufs=agg_bufs, space="PSUM") as psum_agg,
            tc.tile_pool(name="psum_mm", bufs=2, space="PSUM") as psum_mm,
            tc.tile_pool(name="dram", bufs=2, space="DRAM") as dram,
        ):
            # --- constants, loaded once
            iota = cpool.tile([128, 128], f32)
            nc.sync.dma_start(out=iota[:], in_=iota_in[:])
            idx_lo = cpool.tile([128, max(NT_lo * 8, 1)], i16)
            nc.sync.dma_start(out=idx_lo[:], in_=idx_lo_in[:])
            idx_hi = cpool.tile([128, max(NT_hi * 8, 1)], i16)
            nc.sync.dma_start(out=idx_hi[:], in_=idx_hi_in[:])
            dloc_lo = cpool.tile([128, max(NT_lo, 1)], f32)
            nc.sync.dma_start(out=dloc_lo[:], in_=dloc_lo_in[:])
            norm_lo = cpool.tile([128, max(NT_lo, 1)], f32)
            nc.sync.dma_start(out=norm_lo[:], in_=norm_lo_in[:])
            dloc_hi = cpool.tile([128, max(NT_hi, 1)], f32)
            nc.sync.dma_start(out=dloc_hi[:], in_=dloc_hi_in[:])
            norm_hi = cpool.tile([128, max(NT_hi, 1)], f32)
            nc.sync.dma_start(out=norm_hi[:], in_=norm_hi_in[:])
            Wt, bt = [], []
            for i in range(n_layers):
                w = cpool.tile([FEAT, fos[i]], f16, tag=f"W{i}")
                nc.sync.dma_start(out=w[:], in_=W_in[i][:])
                Wt.append(w)
                b = cpool.tile([128, fos[i]], f32, tag=f"b{i}")
                nc.sync.dma_start(out=b[:], in_=b_in[i][:])
                bt.append(b)

            # SBUF cache for S tiles reused across layers/reps
            nsc_lo = min(s_cache_tiles, NT_lo)
            nsc_hi = min(s_cache_tiles, NT_hi)
            s_cache = {}
            if nsc_lo + nsc_hi:
                sc = cpool.tile([128, (nsc_lo + nsc_hi) * 128], f16, tag="scache")
                for t in range(nsc_lo):
                    s_cache[(0, t)] = sc[:, t * 128:(t + 1) * 128]
                for t in range(nsc_hi):
                    s_cache[(1, t)] = sc[:, (nsc_lo + t) * 128:(nsc_lo + t + 1) * 128]
            s_built = set()

            def get_S(half, col, dl, nm):
                """S tile for global tile index `col` of half (0=lo,1=hi)."""
                key = (half, col)
                if key in s_cache:
                    if key not in s_built:
                        s_built.add(key)
                        S = s_cache[key]
                        nc.vector.scalar_tensor_tensor(
                            S, iota[:], dl[:, col:col + 1],
                            nm[:, col:col + 1].broadcast_to([128, 128]),
                            mybir.AluOpType.is_equal, mybir.AluOpType.mult)
                    return s_cache[key]
                if probe == "noS":
                    if "hoisted" not in s_cache:
                        S = spool.tile([128, 128], f16, tag="S")
                        nc.vector.tensor_scalar(
                            S[:], iota[:], dl[:, col:col + 1], nm[:, col:col + 1],
                            mybir.AluOpType.is_equal, mybir.AluOpType.mult)
                        s_cache["hoisted"] = S[:]
                    return s_cache["hoisted"]
                S = spool.tile([128, 128], f16, tag="S")
                if probe == "sconst":
                    nc.vector.tensor_scalar(
                        S[:], iota[:], 3.0, 0.5,
                        mybir.AluOpType.is_equal, mybir.AluOpType.mult)
                elif probe == "tt2":
                    nc.vector.tensor_tensor(
                        S[:], iota[:], dl[:, col:col + 1].broadcast_to([128, 128]),
                        mybir.AluOpType.is_equal)
                elif probe == "tsp":
                    nc.vector.tensor_scalar(
                        S[:], iota[:], dl[:, col:col + 1], nm[:, col:col + 1],
                        mybir.AluOpType.is_equal, mybir.AluOpType.mult)
                else:
                    # (iota == dl) * nm via scalar_tensor_tensor: one DVE op,
                    # avoids the dual-pointer TensorScalar form (slow on HW).
                    nc.vector.scalar_tensor_tensor(
                        S[:], iota[:], dl[:, col:col + 1],
                        nm[:, col:col + 1].broadcast_to([128, 128]),
                        mybir.AluOpType.is_equal, mybir.AluOpType.mult)
                return S[:]

            gq = [0]

            def emit_gather(M, src_ap, idx_sb, t0, nt):
                if probe == "nogather":
                    nc.sync.dma_start(out=M[:],
                                      in_=src_ap[0:128 * nt, :].rearrange(
                                          "(p t) f -> p t f", p=128))
                    return
                step = nt if not max_gather_idx else max(1, max_gather_idx // 128)
                for s0 in range(0, nt, step):
                    sn = min(step, nt - s0)
                    o16 = (t0 + s0) * 8
                    gq[0] = (gq[0] + 1) % swdge_queues
                    nc.gpsimd.dma_gather(
                        out_ap=M[:, s0:s0 + sn, :], in_ap=src_ap,
                        idxs_ap=idx_sb[:, o16:o16 + sn * 8],
                        num_idxs=128 * sn, num_idxs_reg=128 * sn,
                        elem_size=FEAT, single_packet=single_packet,
                        queue_num=gq[0])

            for rep in range(reps):
                # --- layer 0 input: cast x_shard to fp16 (split A/B bounce)
                ag_in_a = dram.tile([ROWS_A, FEAT], f16, tag="ag_in_a")
                ag_in_b = dram.tile([S_pc - ROWS_A, FEAT], f16, tag="ag_in_b")
                for b in range(nblocks):
                    r0, r1 = b * 128, min((b + 1) * 128, S_pc)
                    rows = r1 - r0
                    xt = sb.tile([128, FEAT], f32, tag="xcast_in")
                    nc.sync.dma_start(out=xt[:rows, :], in_=x_shard[r0:r1, :])
                    xh = sb.tile([128, FEAT], f16, tag="xcast_out")
                    nc.vector.tensor_copy(xh[:rows, :], xt[:rows, :])
                    if b < split_blk:
                        nc.sync.dma_start(out=ag_in_a[r0:r1, :], in_=xh[:rows, :])
                    else:
                        nc.sync.dma_start(
                            out=ag_in_b[r0 - ROWS_A:r1 - ROWS_A, :],
                            in_=xh[:rows, :])

                for l in range(n_layers):
                    fo = fos[l]
                    # --- allgather: region A (blocks < split_blk), region B
                    asp = "Shared" if (shared_cc and not model) else "Local"
                    tbl_a = dram.tile([NA, FEAT], f16, tag="tbl_a",
                                      addr_space=asp)
                    tbl_b = dram.tile([NB, FEAT], f16, tag="tbl_b",
                                      addr_space=asp)
                    if model:
                        nc.sync.dma_start(out=tbl_a[0:ROWS_A, :], in_=ag_in_a[:])
                        nc.sync.dma_start(out=tbl_b[0:S_pc - ROWS_A, :], in_=ag_in_b[:])
                    else:
                        nc.gpsimd.collective_compute(
                            "AllGather", mybir.AluOpType.bypass,
                            replica_groups=rg,
                            ins=[ag_in_a[:].opt()], outs=[tbl_a[:].opt()])
                        nc.gpsimd.collective_compute(
                            "AllGather", mybir.AluOpType.bypass,
                            replica_groups=rg,
                            ins=[ag_in_b[:].opt()], outs=[tbl_b[:].opt()])
                    if l + 1 < n_layers:
                        ag_in_a = dram.tile([ROWS_A, FEAT], f16, tag="ag_in_a")
                        ag_in_b = dram.tile([S_pc - ROWS_A, FEAT], f16, tag="ag_in_b")

                    for chunk in chunks:
                        ctlo = int(sum(T_lo[b] for b in chunk))
                        cthi = int(sum(T_hi[b] for b in chunk))
                        M_lo = M_hi = None
                        if ctlo:
                            M_lo = mbuf.tile([128, ctlo, 128], f16, tag="Mlo")
                            emit_gather(M_lo, tbl_a[:], idx_lo,
                                        int(cum_lo[chunk[0]]), ctlo)
                        if cthi:
                            M_hi = mbuf.tile([128, cthi, 128], f16, tag="Mhi")
                            emit_gather(M_hi, tbl_b[:], idx_hi,
                                        int(cum_hi[chunk[0]]), cthi)
                        for b in chunk:
                            tiles = (
                                [(0, M_lo, cum_lo[b] - cum_lo[chunk[0]] + k,
                                  cum_lo[b] + k, dloc_lo, norm_lo)
                                 for k in range(T_lo[b])] +
                                [(1, M_hi, cum_hi[b] - cum_hi[chunk[0]] + k,
                                  cum_hi[b] + k, dloc_hi, norm_hi)
                                 for k in range(T_hi[b])])
                            agg = psum_agg.tile([128, 128], f32)
                            for j, (half, Mt, sl, col, dl, nm) in enumerate(tiles):
                                S = get_S(half, int(col), dl, nm)
                                nc.tensor.matmul(agg[:], Mt[:, sl, :], S,
                                                 start=(j == 0),
                                                 stop=(j == len(tiles) - 1))
                            aggT = sb.tile([128, 128], f16, tag="aggT")
                            nc.scalar.copy(aggT[:], agg[:])
                            mm = psum_mm.tile([128, fo], f32, tag="mm")
                            nc.tensor.matmul(mm[:], aggT[:], Wt[l][:],
                                             start=True, stop=True)
                            r0, r1 = b * 128, min((b + 1) * 128, S_pc)
                            rows = r1 - r0
                            if l + 1 < n_layers:
                                hs = sb.tile([128, fo], f16, tag="hsum")
                                nc.vector.tensor_tensor(
                                    hs[:], mm[:], bt[l][:], mybir.AluOpType.add)
                                h = sb.tile([128, fo], f16, tag="hout")
                                nc.scalar.activation(
                                    h[:], hs[:], mybir.ActivationFunctionType.Relu)
                                if b < split_blk:
                                    nc.sync.dma_start(out=ag_in_a[r0:r1, :],
                                                      in_=h[:rows, :])
                                else:
                                    nc.sync.dma_start(
                                        out=ag_in_b[r0 - ROWS_A:r1 - ROWS_A, :],
                                        in_=h[:rows, :])
                            else:
                                o = sb.tile([128, fo], f32, tag="oout")
                                nc.vector.tensor_tensor(
                                    o[:], mm[:], bt[l][:], mybir.AluOpType.add)
                                nc.sync.dma_start(out=y_out[r0:r1, :],
                                                  in_=o[:rows, :])
    if compile:
        nc.compile()
    return nc


# ---------------------------------------------------------------- entry point

_CACHE = {}


def kernel(x, edge_index, W1, b1, W2, b2, W3, b3):
    import sys
    if "/opt/trn_rl_repo" not in sys.path:
        sys.path.insert(0, "/opt/trn_rl_repo")
    from concourse import bass_utils

    x = np.asarray(x)
    edge_index = np.asarray(edge_index)
    Ws = [np.asarray(W1), np.asarray(W2), np.asarray(W3)]
    bs = [np.asarray(b1), np.asarray(b2), np.asarray(b3)]
    n = x.shape[0]

    key = (n, edge_index.shape[1])
    if key in _CACHE and np.array_equal(_CACHE[key][0], edge_index):
        _, sched, per_core, nc = _CACHE[key]
    else:
        sched, per_core = preprocess(edge_index, n, N_CORES)
        nc = build_nc(sched, fos=(W1.shape[1], W2.shape[1], W3.shape[1]),
                      s_cache_tiles=SCACHE_TILES)
        _CACHE[key] = (edge_index.copy(), sched, per_core, nc)

    in_maps = make_inputs(sched, per_core, x, Ws, bs)
    res = bass_utils.run_bass_kernel_spmd(nc, in_maps,
                                          core_ids=list(range(N_CORES)))
    out = np.concatenate([res.results[r]["y"] for r in range(N_CORES)], axis=0)
    return out.astype(np.float32)


SCACHE_TILES = 0  # tuned below; number of per-half S tiles cached in SBUF



# revision 9
# speedup vs baseline: 1.3742x; 1.1473x over previous
"""3-layer GCN (PyG GCNConv semantics) on 8 Trainium2 NeuronCores.

Contract: kernel(**inputs) takes the FULL inputs (x [50000,128] f32,
edge_index [2,800000] int, W1/b1/W2/b2/W3/b3) and returns the FULL
output [50000, 64] f32.

Sharding: nodes are partitioned across the 8 cores by destination
(6250 rows each).  Per layer, each core casts its local rows to fp16 and
an AllGather replicates the full node-feature table to every core's DRAM.
Each core then computes its destination shard:

    out[d,:] = (sum_{edges (s,d)} norm_e * h[s,:]) @ W + b     (+ReLU)

with norm_e = dinv[s]*dinv[d] precomputed per edge on the host (self-loop
edges included).  The sparse aggregation runs as: dma_gather of the 128
source rows of each edge tile (fp16, 256B rows), a one-hot indicator
S[e,d] = norm_e * (d == dst_local_e) built in a single DVE tensor_scalar
(iota is_equal dstloc, then mult norm), and a PE matmul
aggT[f,d] += M[e,f].T @ S[e,d] accumulated in PSUM over the ~17 edge
tiles of each 128-dst block.  dma_gather indices are int16, so the table
is addressed as two halves (<25600 / >=25600 local-row split), which also
lets each half's AllGather fire as soon as the first/second half of the
blocks finished (the per-edge metadata bakes the node->table-row map).
"""

import numpy as np

FEAT = 128
N_CORES = 8
SPLIT_BLK = 25        # blocks 0..24 -> table region A, 25.. -> region B
CHUNK_BLOCKS = 2


# ---------------------------------------------------------------- host side

def preprocess(edge_index: np.ndarray, n_nodes: int, n_cores: int = N_CORES,
               chunk_blocks: int = CHUNK_BLOCKS):
    """Uniform SPMD tile schedule + per-core gather/metadata arrays."""
    src = np.asarray(edge_index[0], dtype=np.int64)
    dst = np.asarray(edge_index[1], dtype=np.int64)
    loops = np.arange(n_nodes, dtype=np.int64)
    s = np.concatenate([src, loops])
    d = np.concatenate([dst, loops])
    deg = np.bincount(d, minlength=n_nodes).astype(np.float64)  # >= 1
    dinv = 1.0 / np.sqrt(deg)
    norm = (dinv[s] * dinv[d]).astype(np.float32)

    S_pc = n_nodes // n_cores
    assert S_pc * n_cores == n_nodes
    nblocks = (S_pc + 127) // 128
    split_blk = min(SPLIT_BLK, (nblocks + 1) // 2)
    ROWS_A = min(split_blk * 128, S_pc)      # local rows in table region A
    ROWS_B = S_pc - ROWS_A
    assert n_cores * ROWS_A < 32768 and n_cores * ROWS_B < 32768

    # node -> table row (region, row): node = q*S_pc + p
    q = s // S_pc
    p = s - q * S_pc
    in_a = p < ROWS_A
    trow = np.where(in_a, q * ROWS_A + p, q * ROWS_B + (p - ROWS_A))

    core_of = d // S_pc
    dloc = d - core_of * S_pc
    blk = dloc // 128
    dst_local = (dloc - blk * 128).astype(np.int32)
    is_hi = (~in_a).astype(np.int64)

    counts = np.zeros((n_cores, nblocks, 2), dtype=np.int64)
    np.add.at(counts, (core_of, blk, is_hi), 1)
    T_lo = -(-counts[:, :, 0].max(axis=0) // 128)
    T_hi = -(-counts[:, :, 1].max(axis=0) // 128)

    order = np.lexsort((is_hi, blk, core_of))
    trow_o, norm_o, dl_o = trow[order], norm[order], dst_local[order]
    co_o, blk_o, hi_o = core_of[order], blk[order], is_hi[order]

    n_tiles_lo = int(T_lo.sum())
    n_tiles_hi = int(T_hi.sum())
    cum_lo = np.concatenate([[0], np.cumsum(T_lo)])
    cum_hi = np.concatenate([[0], np.cumsum(T_hi)])

    key = (co_o * nblocks + blk_o) * 2 + hi_o
    bounds = np.searchsorted(key, np.arange(n_cores * nblocks * 2 + 1))
    per_core = []
    for r in range(n_cores):
        idx_lo = np.zeros(128 * n_tiles_lo, dtype=np.int16)
        idx_hi = np.zeros(128 * n_tiles_hi, dtype=np.int16)
        dloc_lo = np.full((128, max(n_tiles_lo, 1)), -1.0, dtype=np.float32)
        norm_lo = np.zeros((128, max(n_tiles_lo, 1)), dtype=np.float32)
        dloc_hi = np.full((128, max(n_tiles_hi, 1)), -1.0, dtype=np.float32)
        norm_hi = np.zeros((128, max(n_tiles_hi, 1)), dtype=np.float32)
        for b in range(nblocks):
            for h in range(2):
                k = (r * nblocks + b) * 2 + h
                lo_, hi_ = bounds[k], bounds[k + 1]
                cnt = hi_ - lo_
                t0 = cum_lo[b] if h == 0 else cum_hi[b]
                iarr = idx_lo if h == 0 else idx_hi
                darr = dloc_lo if h == 0 else dloc_hi
                narr = norm_lo if h == 0 else norm_hi
                iarr[128 * t0: 128 * t0 + cnt] = trow_o[lo_:hi_].astype(np.int16)
                e = np.arange(cnt)
                darr[e % 128, t0 + e // 128] = dl_o[lo_:hi_]
                narr[e % 128, t0 + e // 128] = norm_o[lo_:hi_]

        def wrap(a):  # [n] -> [128, n//16]; idx i at [i%16 + 16k, i//16]
            n = len(a)
            if n == 0:
                return np.zeros((128, 1), dtype=np.int16)
            w = a.reshape(n // 16, 16).T
            return np.tile(w, (8, 1)).copy()

        per_core.append(dict(
            idx_lo=wrap(idx_lo), idx_hi=wrap(idx_hi),
            dloc_lo=dloc_lo, norm_lo=norm_lo,
            dloc_hi=dloc_hi, norm_hi=norm_hi,
        ))

    chunks = [list(range(c, min(c + chunk_blocks, nblocks)))
              for c in range(0, nblocks, chunk_blocks)]
    sched = dict(
        n_nodes=n_nodes, n_cores=n_cores, S_pc=S_pc, nblocks=nblocks,
        split_blk=split_blk, ROWS_A=ROWS_A, ROWS_B=ROWS_B,
        T_lo=T_lo.astype(int), T_hi=T_hi.astype(int),
        cum_lo=cum_lo.astype(int), cum_hi=cum_hi.astype(int),
        n_tiles_lo=n_tiles_lo, n_tiles_hi=n_tiles_hi, chunks=chunks,
    )
    return sched, per_core


def make_inputs(sched, per_core, x, Ws, bs):
    n_cores, S_pc = sched["n_cores"], sched["S_pc"]
    iota = np.tile(np.arange(128, dtype=np.float32)[None, :], (128, 1))
    in_maps = []
    for r in range(n_cores):
        m = dict(
            x_shard=np.ascontiguousarray(x[r * S_pc:(r + 1) * S_pc]).astype(np.float32),
            iota=iota,
            idx_lo=per_core[r]["idx_lo"], idx_hi=per_core[r]["idx_hi"],
            dloc_lo=per_core[r]["dloc_lo"], norm_lo=per_core[r]["norm_lo"],
            dloc_hi=per_core[r]["dloc_hi"], norm_hi=per_core[r]["norm_hi"],
        )
        for i, (W, b) in enumerate(zip(Ws, bs)):
            m[f"W{i}"] = np.asarray(W).astype(np.float16)
            m[f"b{i}"] = np.tile(np.asarray(b, dtype=np.float32)[None, :], (128, 1))
        in_maps.append(m)
    return in_maps


# ---------------------------------------------------------------- device side

def build_nc(sched, fos=(128, 128, 64), n_cores=None, model=False,
             compile=True, single_packet=False, max_gather_idx=None,
             scratch=None, reps=1, s_cache_tiles=0, probe=None,
             swdge_queues=4, agg_bufs=4, shared_cc=True, fast_s=True):
    """model=True: single-core cost-model variant (AllGather replaced by a
    local DMA) for TimelineSim.  reps>1 replicates the whole pipeline for
    on-hardware delta timing.  s_cache_tiles: number of leading S tiles
    (per half, by global tile index) kept in SBUF and built only on the
    first layer of the first rep."""
    import concourse.bacc as bacc
    import concourse.tile as tile
    import concourse.mybir as mybir

    f16, f32, i16 = mybir.dt.float16, mybir.dt.float32, mybir.dt.int16
    N, S_pc = sched["n_nodes"], sched["S_pc"]
    nblocks = sched["nblocks"]
    ROWS_A = sched["ROWS_A"]
    split_blk = sched["split_blk"]
    T_lo, T_hi = sched["T_lo"], sched["T_hi"]
    cum_lo, cum_hi = sched["cum_lo"], sched["cum_hi"]
    NT_lo, NT_hi = sched["n_tiles_lo"], sched["n_tiles_hi"]
    chunks = sched["chunks"]
    n_cores = n_cores or sched["n_cores"]
    n_layers = len(fos)
    NA = n_cores * ROWS_A            # rows in table region A
    NB = N - NA
    assert NB > 0, "need a non-empty table region B"

    kw = {}
    if scratch:
        kw["dynamic_dma_scratch_size"] = scratch
    if swdge_queues > 1:
        kw["num_swdge_queues"] = swdge_queues
    nc = bacc.Bacc("TRN2", target_bir_lowering=False, debug=False,
                   num_devices=n_cores, **kw)

    x_shard = nc.dram_tensor("x_shard", [S_pc, FEAT], f32, kind="ExternalInput")
    iota_in = nc.dram_tensor("iota", [128, 128], f32, kind="ExternalInput")
    idx_lo_in = nc.dram_tensor("idx_lo", [128, max(NT_lo * 8, 1)], i16, kind="ExternalInput")
    idx_hi_in = nc.dram_tensor("idx_hi", [128, max(NT_hi * 8, 1)], i16, kind="ExternalInput")
    dloc_lo_in = nc.dram_tensor("dloc_lo", [128, max(NT_lo, 1)], f32, kind="ExternalInput")
    norm_lo_in = nc.dram_tensor("norm_lo", [128, max(NT_lo, 1)], f32, kind="ExternalInput")
    dloc_hi_in = nc.dram_tensor("dloc_hi", [128, max(NT_hi, 1)], f32, kind="ExternalInput")
    norm_hi_in = nc.dram_tensor("norm_hi", [128, max(NT_hi, 1)], f32, kind="ExternalInput")
    W_in = [nc.dram_tensor(f"W{i}", [FEAT, fos[i]], f16, kind="ExternalInput")
            for i in range(n_layers)]
    b_in = [nc.dram_tensor(f"b{i}", [128, fos[i]], f32, kind="ExternalInput")
            for i in range(n_layers)]
    y_out = nc.dram_tensor("y", [S_pc, fos[-1]], f32, kind="ExternalOutput")

    rg = [list(range(n_cores))]

    with tile.TileContext(nc) as tc:
        with (
            tc.tile_pool(name="const", bufs=1) as cpool,
            tc.tile_pool(name="sb", bufs=3) as sb,
            tc.tile_pool(name="mbuf", bufs=3) as mbuf,
            tc.tile_pool(name="spool", bufs=6) as spool,
            tc.tile_pool(name="psum_agg", bufs=agg_bufs, space="PSUM") as psum_agg,
            tc.tile_pool(name="psum_mm", bufs=2, space="PSUM") as psum_mm,
            tc.tile_pool(name="dram", bufs=2, space="DRAM") as dram,
        ):
            # --- constants, loaded once
            iota = cpool.tile([128, 128], f32)
            nc.sync.dma_start(out=iota[:], in_=iota_in[:])
            idx_lo = cpool.tile([128, max(NT_lo * 8, 1)], i16)
            nc.sync.dma_start(out=idx_lo[:], in_=idx_lo_in[:])
            idx_hi = cpool.tile([128, max(NT_hi * 8, 1)], i16)
            nc.sync.dma_start(out=idx_hi[:], in_=idx_hi_in[:])
            dloc_lo = cpool.tile([128, max(NT_lo, 1)], f32)
            nc.sync.dma_start(out=dloc_lo[:], in_=dloc_lo_in[:])
            norm_lo = cpool.tile([128, max(NT_lo, 1)], f32)
            nc.sync.dma_start(out=norm_lo[:], in_=norm_lo_in[:])
            dloc_hi = cpool.tile([128, max(NT_hi, 1)], f32)
            nc.sync.dma_start(out=dloc_hi[:], in_=dloc_hi_in[:])
            norm_hi = cpool.tile([128, max(NT_hi, 1)], f32)
            nc.sync.dma_start(out=norm_hi[:], in_=norm_hi_in[:])
            Wt, bt = [], []
            for i in range(n_layers):
                w = cpool.tile([FEAT, fos[i]], f16, tag=f"W{i}")
                nc.sync.dma_start(out=w[:], in_=W_in[i][:])
                Wt.append(w)
                b = cpool.tile([128, fos[i]], f32, tag=f"b{i}")
                nc.sync.dma_start(out=b[:], in_=b_in[i][:])
                bt.append(b)

            # SBUF cache for S tiles reused across layers/reps
            nsc_lo = min(s_cache_tiles, NT_lo)
            nsc_hi = min(s_cache_tiles, NT_hi)
            s_cache = {}
            if nsc_lo + nsc_hi:
                sc = cpool.tile([128, (nsc_lo + nsc_hi) * 128], f16, tag="scache")
                for t in range(nsc_lo):
                    s_cache[(0, t)] = sc[:, t * 128:(t + 1) * 128]
                for t in range(nsc_hi):
                    s_cache[(1, t)] = sc[:, (nsc_lo + t) * 128:(nsc_lo + t + 1) * 128]
            s_built = set()

            def get_S(half, col, dl, nm):
                """S tile for global tile index `col` of half (0=lo,1=hi)."""
                key = (half, col)
                if key in s_cache:
                    if key not in s_built:
                        s_built.add(key)
                        S = s_cache[key]
                        nc.vector.scalar_tensor_tensor(
                            S, iota[:], dl[:, col:col + 1],
                            nm[:, col:col + 1].broadcast_to([128, 128]),
                            mybir.AluOpType.is_equal, mybir.AluOpType.mult)
                    return s_cache[key]
                if probe == "noS":
                    if "hoisted" not in s_cache:
                        S = spool.tile([128, 128], f16, tag="S")
                        nc.vector.tensor_scalar(
                            S[:], iota[:], dl[:, col:col + 1], nm[:, col:col + 1],
                            mybir.AluOpType.is_equal, mybir.AluOpType.mult)
                        s_cache["hoisted"] = S[:]
                    return s_cache["hoisted"]
                S = spool.tile([128, 128], f16, tag="S")
                if probe == "sconst":
                    nc.vector.tensor_scalar(
                        S[:], iota[:], 3.0, 0.5,
                        mybir.AluOpType.is_equal, mybir.AluOpType.mult)
                elif probe == "tt2":
                    nc.vector.tensor_tensor(
                        S[:], iota[:], dl[:, col:col + 1].broadcast_to([128, 128]),
                        mybir.AluOpType.is_equal)
                elif probe == "tsp":
                    nc.vector.tensor_scalar(
                        S[:], iota[:], dl[:, col:col + 1], nm[:, col:col + 1],
                        mybir.AluOpType.is_equal, mybir.AluOpType.mult)
                else:
                    # (iota == dl) * nm via scalar_tensor_tensor: one DVE op,
                    # avoids the dual-pointer TensorScalar form (slow on HW).
                    nc.vector.scalar_tensor_tensor(
                        S[:], iota[:], dl[:, col:col + 1],
                        nm[:, col:col + 1].broadcast_to([128, 128]),
                        mybir.AluOpType.is_equal, mybir.AluOpType.mult)
                return S[:]

            gq = [0]

            def emit_gather(M, src_ap, idx_sb, t0, nt):
                if probe == "nogather":
                    nc.sync.dma_start(out=M[:],
                                      in_=src_ap[0:128 * nt, :].rearrange(
                                          "(p t) f -> p t f", p=128))
                    return
                step = nt if not max_gather_idx else max(1, max_gather_idx // 128)
                for s0 in range(0, nt, step):
                    sn = min(step, nt - s0)
                    o16 = (t0 + s0) * 8
                    gq[0] = (gq[0] + 1) % swdge_queues
                    nc.gpsimd.dma_gather(
                        out_ap=M[:, s0:s0 + sn, :], in_ap=src_ap,
                        idxs_ap=idx_sb[:, o16:o16 + sn * 8],
                        num_idxs=128 * sn, num_idxs_reg=128 * sn,
                        elem_size=FEAT, single_packet=single_packet,
                        queue_num=gq[0])

            for rep in range(reps):
                # --- layer 0 input: cast x_shard to fp16 (split A/B bounce)
                ag_in_a = dram.tile([ROWS_A, FEAT], f16, tag="ag_in_a")
                ag_in_b = dram.tile([S_pc - ROWS_A, FEAT], f16, tag="ag_in_b")
                for b in range(nblocks):
                    r0, r1 = b * 128, min((b + 1) * 128, S_pc)
                    rows = r1 - r0
                    xt = sb.tile([128, FEAT], f32, tag="xcast_in")
                    nc.sync.dma_start(out=xt[:rows, :], in_=x_shard[r0:r1, :])
                    xh = sb.tile([128, FEAT], f16, tag="xcast_out")
                    nc.vector.tensor_copy(xh[:rows, :], xt[:rows, :])
                    if b < split_blk:
                        nc.sync.dma_start(out=ag_in_a[r0:r1, :], in_=xh[:rows, :])
                    else:
                        nc.sync.dma_start(
                            out=ag_in_b[r0 - ROWS_A:r1 - ROWS_A, :],
                            in_=xh[:rows, :])

                for l in range(n_layers):
                    fo = fos[l]
                    # --- allgather: region A (blocks < split_blk), region B
                    asp = "Shared" if (shared_cc and not model) else "Local"
                    tbl_a = dram.tile([NA, FEAT], f16, tag="tbl_a",
                                      addr_space=asp)
                    tbl_b = dram.tile([NB, FEAT], f16, tag="tbl_b",
                                      addr_space=asp)
                    if model:
                        nc.sync.dma_start(out=tbl_a[0:ROWS_A, :], in_=ag_in_a[:])
                        nc.sync.dma_start(out=tbl_b[0:S_pc - ROWS_A, :], in_=ag_in_b[:])
                    else:
                        nc.gpsimd.collective_compute(
                            "AllGather", mybir.AluOpType.bypass,
                            replica_groups=rg,
                            ins=[ag_in_a[:].opt()], outs=[tbl_a[:].opt()])
                        nc.gpsimd.collective_compute(
                            "AllGather", mybir.AluOpType.bypass,
                            replica_groups=rg,
                            ins=[ag_in_b[:].opt()], outs=[tbl_b[:].opt()])
                    if l + 1 < n_layers:
                        ag_in_a = dram.tile([ROWS_A, FEAT], f16, tag="ag_in_a")
                        ag_in_b = dram.tile([S_pc - ROWS_A, FEAT], f16, tag="ag_in_b")

                    for chunk in chunks:
                        ctlo = int(sum(T_lo[b] for b in chunk))
                        cthi = int(sum(T_hi[b] for b in chunk))
                        M_lo = M_hi = None
                        Sg = [None, None]
                        if ctlo:
                            M_lo = mbuf.tile([128, ctlo, 128], f16, tag="Mlo")
                            emit_gather(M_lo, tbl_a[:], idx_lo,
                                        int(cum_lo[chunk[0]]), ctlo)
                        if cthi:
                            M_hi = mbuf.tile([128, cthi, 128], f16, tag="Mhi")
                            emit_gather(M_hi, tbl_b[:], idx_hi,
                                        int(cum_hi[chunk[0]]), cthi)
                        if fast_s:
                            # one-hot S tiles for the whole chunk in ONE DVE
                            # op per half; norm folded into M (one in-place
                            # broadcast multiply per half).
                            for h, (ct, Mt, dl, nm, c0) in enumerate((
                                    (ctlo, M_lo, dloc_lo, norm_lo,
                                     int(cum_lo[chunk[0]])),
                                    (cthi, M_hi, dloc_hi, norm_hi,
                                     int(cum_hi[chunk[0]])))):
                                if not ct:
                                    continue
                                tag = f"Sg{h}"
                                Sg[h] = spool.tile([128, ct, 128], f16,
                                                   name=tag, tag=tag, bufs=3)
                                nc.vector.tensor_tensor(
                                    Sg[h][:],
                                    iota[:].unsqueeze(1).broadcast_to(
                                        [128, ct, 128]),
                                    dl[:, c0:c0 + ct].unsqueeze(2).broadcast_to(
                                        [128, ct, 128]),
                                    mybir.AluOpType.is_equal)
                                nc.vector.tensor_tensor(
                                    Mt[:], Mt[:],
                                    nm[:, c0:c0 + ct].unsqueeze(2).broadcast_to(
                                        [128, ct, 128]),
                                    mybir.AluOpType.mult)
                        for b in chunk:
                            tiles = (
                                [(0, M_lo, cum_lo[b] - cum_lo[chunk[0]] + k,
                                  cum_lo[b] + k, dloc_lo, norm_lo)
                                 for k in range(T_lo[b])] +
                                [(1, M_hi, cum_hi[b] - cum_hi[chunk[0]] + k,
                                  cum_hi[b] + k, dloc_hi, norm_hi)
                                 for k in range(T_hi[b])])
                            agg = psum_agg.tile([128, 128], f32)
                            for j, (half, Mt, sl, col, dl, nm) in enumerate(tiles):
                                if fast_s:
                                    S = Sg[half][:, sl, :]
                                else:
                                    S = get_S(half, int(col), dl, nm)
                                nc.tensor.matmul(agg[:], Mt[:, sl, :], S,
                                                 start=(j == 0),
                                                 stop=(j == len(tiles) - 1))
                            aggT = sb.tile([128, 128], f16, tag="aggT")
                            nc.scalar.copy(aggT[:], agg[:])
                            mm = psum_mm.tile([128, fo], f32, tag="mm")
                            nc.tensor.matmul(mm[:], aggT[:], Wt[l][:],
                                             start=True, stop=True)
                            r0, r1 = b * 128, min((b + 1) * 128, S_pc)
                            rows = r1 - r0
                            if l + 1 < n_layers:
                                hs = sb.tile([128, fo], f16, tag="hsum")
                                nc.vector.tensor_tensor(
                                    hs[:], mm[:], bt[l][:], mybir.AluOpType.add)
                                h = sb.tile([128, fo], f16, tag="hout")
                                nc.scalar.activation(
                                    h[:], hs[:], mybir.ActivationFunctionType.Relu)
                                if b < split_blk:
                                    nc.sync.dma_start(out=ag_in_a[r0:r1, :],
                                                      in_=h[:rows, :])
                                else:
                                    nc.sync.dma_start(
                                        out=ag_in_b[r0 - ROWS_A:r1 - ROWS_A, :],
                                        in_=h[:rows, :])
                            else:
                                o = sb.tile([128, fo], f32, tag="oout")
                                nc.vector.tensor_tensor(
                                    o[:], mm[:], bt[l][:], mybir.AluOpType.add)
                                nc.sync.dma_start(out=y_out[r0:r1, :],
                                                  in_=o[:rows, :])
    if compile:
        nc.compile()
    return nc


# ---------------------------------------------------------------- entry point

_CACHE = {}


def kernel(x, edge_index, W1, b1, W2, b2, W3, b3):
    import sys
    if "/opt/trn_rl_repo" not in sys.path:
        sys.path.insert(0, "/opt/trn_rl_repo")
    from concourse import bass_utils

    x = np.asarray(x)
    edge_index = np.asarray(edge_index)
    Ws = [np.asarray(W1), np.asarray(W2), np.asarray(W3)]
    bs = [np.asarray(b1), np.asarray(b2), np.asarray(b3)]
    n = x.shape[0]

    key = (n, edge_index.shape[1])
    if key in _CACHE and np.array_equal(_CACHE[key][0], edge_index):
        _, sched, per_core, nc = _CACHE[key]
    else:
        sched, per_core = preprocess(edge_index, n, N_CORES)
        nc = build_nc(sched, fos=(W1.shape[1], W2.shape[1], W3.shape[1]),
                      s_cache_tiles=SCACHE_TILES)
        _CACHE[key] = (edge_index.copy(), sched, per_core, nc)

    in_maps = make_inputs(sched, per_core, x, Ws, bs)
    res = bass_utils.run_bass_kernel_spmd(nc, in_maps,
                                          core_ids=list(range(N_CORES)))
    out = np.concatenate([res.results[r]["y"] for r in range(N_CORES)], axis=0)
    return out.astype(np.float32)


SCACHE_TILES = 0  # tuned below; number of per-half S tiles cached in SBUF

